# revision 20
# baseline (speedup 1.0000x reference)
"""DeepSeek-V3 TopK router kernel for 8 Trainium2 NeuronCores.

Strategy (data/sequence parallel per sharding hint):
 - Shard the 16384 tokens across 8 cores (2048 tokens each); replicate the
   router weight + bias.
 - Host-side layout prep: x and w are transposed so the contraction dim
   (hidden) lands on SBUF partitions, scaled by exact powers of two
   (x*2^10, w*2^13) and split into fp16 hi+lo halves. Total DMA bytes are
   unchanged vs fp32 (2+2 bytes/elem) and the per-product error (~2^-21)
   sits below fp32 summation noise, while the PE runs at full 16-bit rate
   (3 passes) instead of native-fp32's 4x cost.
 - Device per core (build_nc_f16): per 128-token tile, 168 accumulating
   fp16 matmuls (lhsT = x^T chunk [128h,128t] stationary, rhs = w^T chunk
   [128h,256e] moving, fp32 PSUM [128t,256e]); hidden dim is DMA'd in
   7 chunks of 8x128 so matmuls start before whole tensors land; sigmoid
   (with 2^-23 descale) on ScalarE; group top-2 / top-4 and masked top-8
   via the DVE max8/max_index ops; score gather by position (iota ==
   index one-hot, fused scalar_tensor_tensor with accumulate — value
   matching would break on bit-equal score ties); normalize with the DVE
   iterative-divide reciprocal.
 - build_nc/make_in_maps keep the plain-fp32 variant (4x PE cost, same
   results) for reference.
"""

import sys

for _p in ("/opt/trn_rl_repo", "/root/.axon_site/_ro/trn_rl_repo"):
    if _p not in sys.path:
        sys.path.append(_p)

import numpy as np

import concourse.bass as bass
import concourse.bacc as bacc
import concourse.mybir as mybir
from concourse.bass import ds
from concourse.tile import TileContext
from concourse.bass_utils import run_bass_kernel_spmd

# Problem constants (hardcoded per contract)
T = 16384          # batch*seq = 4*4096
H = 7168           # hidden
E = 256            # experts
N_CORES = 8
TC = T // N_CORES  # tokens per core = 2048
KH = H // 128      # 56 contraction chunks
NG = 8             # expert groups
GS = E // NG       # group size = 32
TOPK = 8
SCALE = 2.5
TGROUP = 256       # tokens per DMA group (2 matmul subtiles)
CK = 8             # contraction chunks per DMA (k-chunking for overlap)
NCH = KH // CK     # 7 DMA chunks over the hidden dim

F32 = mybir.dt.float32
F32R = mybir.dt.float32r
U32 = mybir.dt.uint32


def build_nc(tc_tokens: int = TC, tgroup: int = TGROUP, debug: bool = False) -> bass.Bass:
    nc = bacc.Bacc(trn_type="TRN2")
    xt = nc.declare_dram_parameter("xt", [H, tc_tokens], F32, isOutput=False)
    wt = nc.declare_dram_parameter("wt", [H, E], F32, isOutput=False)
    biasb = nc.declare_dram_parameter("biasb", [128, E], F32, isOutput=False)
    iotab = nc.declare_dram_parameter("iotab", [128, E], F32, isOutput=False)
    out_logits = nc.declare_dram_parameter("out_logits", [tc_tokens, E], F32, isOutput=True)
    out_idx = nc.declare_dram_parameter("out_idx", [tc_tokens, TOPK], U32, isOutput=True)
    out_w = nc.declare_dram_parameter("out_w", [tc_tokens, TOPK], F32, isOutput=True)
    if debug:
        out_w8 = nc.declare_dram_parameter("out_w8", [tc_tokens, TOPK], F32, isOutput=True)
        out_den = nc.declare_dram_parameter("out_den", [tc_tokens, 2], F32, isOutput=True)
        out_eqs = nc.declare_dram_parameter("out_eqs", [tc_tokens, E], F32, isOutput=True)

    n_groups = tc_tokens // tgroup
    subs = tgroup // 128

    with TileContext(nc) as tc:
        with (
            tc.tile_pool(name="const", bufs=1) as cpool,
            tc.tile_pool(name="xs", bufs=2) as xpool,
            tc.tile_pool(name="ps", bufs=6, space="PSUM") as ppool,
            tc.tile_pool(name="work", bufs=2) as spool,
        ):
            wt_sb = cpool.tile([128, KH, E], F32)
            nc.sync.dma_start(out=wt_sb, in_=wt[:, :].rearrange("(k p) e -> p k e", p=128))
            bias_sb = cpool.tile([128, E], F32)
            nc.sync.dma_start(out=bias_sb, in_=biasb[:, :])
            iota_sb = cpool.tile([128, E], F32)
            nc.sync.dma_start(out=iota_sb, in_=iotab[:, :])
            # Dummy ldweights consuming wt_sb: folds the weight-DMA wait into
            # the PE's clock so the first real matmul carries only the x-tile
            # wait (the LDWEIGHTS ISA slot fits a single sync wait). The real
            # matmuls self-load their stationary operands, overwriting this.
            nc.tensor.ldweights(wt_sb[:, 0, 0:64].bitcast(mybir.dt.bfloat16))

            for g in range(n_groups):
                xg = xpool.tile([128, KH, tgroup], F32, tag="xg")
                nc.sync.dma_start(
                    out=xg,
                    in_=xt[:, ds(g * tgroup, tgroup)].rearrange("(k p) t -> p k t", p=128),
                )
                for sub in range(subs):
                    t0 = g * tgroup + sub * 128
                    lg = ppool.tile([128, E], F32, tag="lg")
                    for k in range(KH):
                        nc.tensor.matmul(
                            lg,
                            xg[:, k, ds(sub * 128, 128)],
                            wt_sb[:, k, :],
                            start=(k == 0),
                            stop=(k == KH - 1),
                        )
                    # router logits: PSUM -> SBUF -> DRAM
                    lsb = spool.tile([128, E], F32, tag="lsb")
                    nc.scalar.copy(lsb, lg)
                    nc.scalar.dma_start(out=out_logits[ds(t0, 128), :], in_=lsb)
                    scores = spool.tile([128, E], F32, tag="scores")
                    nc.scalar.activation(scores, lg, mybir.ActivationFunctionType.Sigmoid)
                    s4c = spool.tile([128, E], F32, tag="s4c")
                    nc.vector.tensor_add(s4c, scores, bias_sb)

                    # per-group top-2 -> group score
                    gtop = spool.tile([128, NG, 8], F32, tag="gtop")
                    for j in range(NG):
                        nc.vector.max(out=gtop[:, j, :], in_=s4c[:, ds(j * GS, GS)])
                    gsum = spool.tile([128, NG], F32, tag="gsum")
                    nc.vector.tensor_add(gsum, gtop[:, :, 0], gtop[:, :, 1])
                    # top-4 groups -> mask
                    gs8 = spool.tile([128, 8], F32, tag="gs8")
                    nc.vector.max(out=gs8, in_=gsum)
                    gmask = spool.tile([128, NG], F32, tag="gmask")
                    nc.vector.tensor_scalar(
                        gmask, gsum, gs8[:, 3:4], None, op0=mybir.AluOpType.is_ge
                    )
                    masked = spool.tile([128, E], F32, tag="masked")
                    for j in range(NG):
                        nc.vector.tensor_scalar_mul(
                            masked[:, ds(j * GS, GS)], s4c[:, ds(j * GS, GS)], gmask[:, j : j + 1]
                        )
                    # masked top-8 with indices
                    top8v = spool.tile([128, 8], F32, tag="top8v")
                    nc.vector.max(out=top8v, in_=masked)
                    idx8 = spool.tile([128, 8], U32, tag="idx8")
                    nc.vector.max_index(idx8, top8v, masked)
                    # gather scores at the top-8 positions by POSITION
                    # (iota == idx_k is a guaranteed one-hot; value-matching
                    # breaks when two experts tie bit-exactly)
                    idx8f = spool.tile([128, TOPK], F32, tag="idx8f")
                    nc.vector.tensor_copy(idx8f, idx8)
                    w8 = spool.tile([128, TOPK], F32, tag="w8")
                    eqs = spool.tile([128, E], F32, tag="eqs")
                    for k in range(TOPK):
                        nc.vector.scalar_tensor_tensor(
                            out=eqs,
                            in0=iota_sb,
                            scalar=idx8f[:, k : k + 1],
                            in1=scores,
                            op0=mybir.AluOpType.is_equal,
                            op1=mybir.AluOpType.mult,
                            accum_out=w8[:, k : k + 1],
                        )
                    denom = spool.tile([128, 1], F32, tag="denom")
                    nc.vector.reduce_sum(denom, w8, axis=mybir.AxisListType.X)
                    rden = spool.tile([128, 1], F32, tag="rden")
                    nc.vector.reciprocal(rden, denom)
                    wout = spool.tile([128, TOPK], F32, tag="wout")
                    nc.vector.tensor_scalar(
                        wout, w8, rden, SCALE,
                        op0=mybir.AluOpType.mult, op1=mybir.AluOpType.mult,
                    )
                    nc.scalar.dma_start(out=out_idx[ds(t0, 128), :], in_=idx8)
                    nc.scalar.dma_start(out=out_w[ds(t0, 128), :], in_=wout)
                    if debug:
                        nc.scalar.dma_start(out=out_w8[ds(t0, 128), :], in_=w8)
                        nc.scalar.dma_start(out=out_den[ds(t0, 128), 0:1], in_=denom)
                        nc.scalar.dma_start(out=out_den[ds(t0, 128), 1:2], in_=rden)
                        nc.scalar.dma_start(out=out_eqs[ds(t0, 128), :], in_=eqs)
    nc.finalize()
    return nc


F16 = mybir.dt.float16
XSCALE = 1024.0       # 2**10: keeps x_lo out of fp16 denormals
WSCALE = 8192.0       # 2**13: keeps w_lo out of fp16 denormals
DESCALE = 1.0 / (XSCALE * WSCALE)  # 2**-23, exact power of two


def build_nc_f16(tc_tokens: int = TC, tgroup: int = TGROUP, debug: bool = False) -> bass.Bass:
    """fp16 hi/lo 3-pass matmul variant: x' = x*2^10 = xhi+xlo (fp16),
    w' = w*2^13 = whi+wlo (fp16); logits' = xhi*whi + xlo*whi + xhi*wlo
    accumulated in fp32 PSUM; descale by 2^-23 (exact) on the way out.
    Error ~2^-21 per product: below fp32 summation noise, at 3x bf16-rate
    PE cost instead of 4x for native fp32."""
    nc = bacc.Bacc(trn_type="TRN2")
    xhi = nc.declare_dram_parameter("xhi", [H, tc_tokens], F16, isOutput=False)
    xlo = nc.declare_dram_parameter("xlo", [H, tc_tokens], F16, isOutput=False)
    whi = nc.declare_dram_parameter("whi", [H, E], F16, isOutput=False)
    wlo = nc.declare_dram_parameter("wlo", [H, E], F16, isOutput=False)
    biasb = nc.declare_dram_parameter("biasb", [128, E], F32, isOutput=False)
    iotab = nc.declare_dram_parameter("iotab", [128, E], F32, isOutput=False)
    out_logits = nc.declare_dram_parameter("out_logits", [tc_tokens, E], F32, isOutput=True)
    out_idx = nc.declare_dram_parameter("out_idx", [tc_tokens, TOPK], U32, isOutput=True)
    out_w = nc.declare_dram_parameter("out_w", [tc_tokens, TOPK], F32, isOutput=True)

    n_groups = tc_tokens // tgroup
    subs = tgroup // 128

    with TileContext(nc) as tc:
        with (
            tc.tile_pool(name="const", bufs=1) as cpool,
            tc.tile_pool(name="xs", bufs=2) as xpool,
            tc.tile_pool(name="ps", bufs=6, space="PSUM") as ppool,
            tc.tile_pool(name="work", bufs=2) as spool,
        ):
            # K is processed in chunks so matmuls can start as soon as the
            # first slices of the weights and x land, instead of waiting for
            # whole-tensor DMAs (~15 MB) to finish.
            # Prologue: interleave weight-chunk and group-0 x-chunk DMAs so
            # the first matmul's pair (whi_c0 + xghi_c0) lands first; bias and
            # iota (needed only by the DVE stage) go last.
            # whi and wlo are packed side by side in one tile so a single
            # N=512 matmul covers both hi passes for a given xhi stationary
            # (halves summed after accumulation; saves 1/3 of the LDWEIGHTS).
            w_ch = []
            for c in range(NCH):
                w_ch.append(cpool.tile([128, CK, 2 * E], F16, tag=f"w{c}", name=f"w_ch{c}"))

            def load_w_chunk(c):
                nc.sync.dma_start(
                    out=w_ch[c][:, :, 0:E],
                    in_=whi[ds(c * CK * 128, CK * 128), :].rearrange("(k p) e -> p k e", p=128))
                nc.sync.dma_start(
                    out=w_ch[c][:, :, E : 2 * E],
                    in_=wlo[ds(c * CK * 128, CK * 128), :].rearrange("(k p) e -> p k e", p=128))

            def load_x_chunk(g, c):
                xh = xpool.tile([128, CK, tgroup], F16, tag=f"xghi{c}")
                nc.sync.dma_start(
                    out=xh,
                    in_=xhi[ds(c * CK * 128, CK * 128), ds(g * tgroup, tgroup)]
                    .rearrange("(k p) t -> p k t", p=128))
                xl = xpool.tile([128, CK, tgroup], F16, tag=f"xglo{c}")
                nc.sync.dma_start(
                    out=xl,
                    in_=xlo[ds(c * CK * 128, CK * 128), ds(g * tgroup, tgroup)]
                    .rearrange("(k p) t -> p k t", p=128))
                return xh, xl

            # Weight chunks run 3 ahead of group-0 x chunks: the PE can
            # start after ~4 chunks (~11us) while the 3-chunk lead absorbs
            # DMA arrival jitter (starting with no lead stutters the PE and
            # re-throttles HAM; waiting for all weights idles it for 37us).
            W_LEAD = 4
            for c in range(W_LEAD):
                load_w_chunk(c)
            g0_chunks = []
            for c in range(NCH):
                g0_chunks.append(load_x_chunk(0, c))
                if c + W_LEAD < NCH:
                    load_w_chunk(c + W_LEAD)
            bias_sb = cpool.tile([128, E], F32)
            nc.sync.dma_start(out=bias_sb, in_=biasb[:, :])
            iota_sb = cpool.tile([128, E], F32)
            nc.sync.dma_start(out=iota_sb, in_=iotab[:, :])

            for g in range(n_groups):
                if g == 0:
                    xgpairs = g0_chunks
                else:
                    xgpairs = [load_x_chunk(g, c) for c in range(NCH)]
                xghi_ch = [p[0] for p in xgpairs]
                xglo_ch = [p[1] for p in xgpairs]
                for sub in range(subs):
                    t0 = g * tgroup + sub * 128
                    ts_ = ds(sub * 128, 128)
                    lg = ppool.tile([128, 2 * E], F32, tag="lg")
                    for k in range(KH):
                        c, kc = divmod(k, CK)
                        nc.tensor.matmul(lg, xghi_ch[c][:, kc, ts_], w_ch[c][:, kc, :],
                                         start=(k == 0), stop=False)
                        nc.tensor.matmul(lg[:, 0:E], xglo_ch[c][:, kc, ts_],
                                         w_ch[c][:, kc, 0:E],
                                         start=False, stop=(k == KH - 1))
                    # halves: lg[:,0:E] = xhi*whi + xlo*whi, lg[:,E:] = xhi*wlo
                    # (one PSUM operand per DVE op: stage the hi-lo half first)
                    lcor = spool.tile([128, E], F32, tag="lcor")
                    nc.scalar.copy(lcor, lg[:, E : 2 * E])
                    lsum = spool.tile([128, E], F32, tag="lsum")
                    nc.vector.tensor_add(lsum, lg[:, 0:E], lcor)
                    # descaled router logits: SBUF -> DRAM
                    lsb = spool.tile([128, E], F32, tag="lsb")
                    nc.scalar.mul(lsb, lsum, DESCALE)
                    nc.scalar.dma_start(out=out_logits[ds(t0, 128), :], in_=lsb)
                    scores = spool.tile([128, E], F32, tag="scores")
                    nc.scalar.activation(scores, lsum, mybir.ActivationFunctionType.Sigmoid,
                                         scale=DESCALE)
                    s4c = spool.tile([128, E], F32, tag="s4c")
                    nc.vector.tensor_add(s4c, scores, bias_sb)

                    gtop = spool.tile([128, NG, 8], F32, tag="gtop")
                    for j in range(NG):
                        nc.vector.max(out=gtop[:, j, :], in_=s4c[:, ds(j * GS, GS)])
                    gsum = spool.tile([128, NG], F32, tag="gsum")
                    nc.vector.tensor_add(gsum, gtop[:, :, 0], gtop[:, :, 1])
                    gs8 = spool.tile([128, 8], F32, tag="gs8")
                    nc.vector.max(out=gs8, in_=gsum)
                    gmask = spool.tile([128, NG], F32, tag="gmask")
                    nc.vector.tensor_scalar(
                        gmask, gsum, gs8[:, 3:4], None, op0=mybir.AluOpType.is_ge
                    )
                    masked = spool.tile([128, E], F32, tag="masked")
                    for j in range(NG):
                        nc.vector.tensor_scalar_mul(
                            masked[:, ds(j * GS, GS)], s4c[:, ds(j * GS, GS)], gmask[:, j : j + 1]
                        )
                    top8v = spool.tile([128, 8], F32, tag="top8v")
                    nc.vector.max(out=top8v, in_=masked)
                    idx8 = spool.tile([128, 8], U32, tag="idx8")
                    nc.vector.max_index(idx8, top8v, masked)
                    idx8f = spool.tile([128, TOPK], F32, tag="idx8f")
                    nc.vector.tensor_copy(idx8f, idx8)
                    w8 = spool.tile([128, TOPK], F32, tag="w8")
                    eqs = spool.tile([128, E], F32, tag="eqs")
                    for k in range(TOPK):
                        nc.vector.scalar_tensor_tensor(
                            out=eqs,
                            in0=iota_sb,
                            scalar=idx8f[:, k : k + 1],
                            in1=scores,
                            op0=mybir.AluOpType.is_equal,
                            op1=mybir.AluOpType.mult,
                            accum_out=w8[:, k : k + 1],
                        )
                    denom = spool.tile([128, 1], F32, tag="denom")
                    nc.vector.reduce_sum(denom, w8, axis=mybir.AxisListType.X)
                    rden = spool.tile([128, 1], F32, tag="rden")
                    nc.vector.reciprocal(rden, denom)
                    wout = spool.tile([128, TOPK], F32, tag="wout")
                    nc.vector.tensor_scalar(
                        wout, w8, rden, SCALE,
                        op0=mybir.AluOpType.mult, op1=mybir.AluOpType.mult,
                    )
                    nc.scalar.dma_start(out=out_idx[ds(t0, 128), :], in_=idx8)
                    nc.scalar.dma_start(out=out_w[ds(t0, 128), :], in_=wout)
    nc.finalize()
    return nc


def make_in_maps_f16(hidden_states, weight, e_score_correction_bias):
    from concurrent.futures import ThreadPoolExecutor

    x = np.ascontiguousarray(np.asarray(hidden_states, dtype=np.float32)).reshape(T, H)
    w = np.asarray(weight, dtype=np.float32)
    b = np.asarray(e_score_correction_bias, dtype=np.float32)
    ws = np.ascontiguousarray(w.T) * np.float32(WSCALE)
    whi = ws.astype(np.float16)
    wlo = (ws - whi.astype(np.float32)).astype(np.float16)
    biasb = np.ascontiguousarray(np.broadcast_to(b[None, :], (128, E)))
    iotab = np.ascontiguousarray(
        np.broadcast_to(np.arange(E, dtype=np.float32)[None, :], (128, E)))
    xt_full = x.T  # view

    def prep(c):
        xs = np.ascontiguousarray(xt_full[:, c * TC : (c + 1) * TC]) * np.float32(XSCALE)
        xhi = xs.astype(np.float16)
        xlo = (xs - xhi.astype(np.float32)).astype(np.float16)
        return {"xhi": xhi, "xlo": xlo, "whi": whi, "wlo": wlo,
                "biasb": biasb, "iotab": iotab}

    with ThreadPoolExecutor(N_CORES) as ex:
        return list(ex.map(prep, range(N_CORES)))


_NC = None


def _get_nc():
    global _NC
    if _NC is None:
        _NC = build_nc_f16()
    return _NC


def make_in_maps(hidden_states, weight, e_score_correction_bias):
    x = np.ascontiguousarray(np.asarray(hidden_states, dtype=np.float32)).reshape(T, H)
    w = np.asarray(weight, dtype=np.float32)
    b = np.asarray(e_score_correction_bias, dtype=np.float32)
    wt = np.ascontiguousarray(w.T)
    biasb = np.ascontiguousarray(np.broadcast_to(b[None, :], (128, E)))
    iotab = np.ascontiguousarray(np.broadcast_to(np.arange(E, dtype=np.float32)[None, :], (128, E)))
    xt_full = x.T  # view
    in_maps = []
    for c in range(N_CORES):
        xt_c = np.ascontiguousarray(xt_full[:, c * TC : (c + 1) * TC])
        in_maps.append({"xt": xt_c, "wt": wt, "biasb": biasb, "iotab": iotab})
    return in_maps


def assemble(results):
    logits = np.concatenate([results[c]["out_logits"] for c in range(N_CORES)], axis=0)
    idx = np.concatenate([results[c]["out_idx"] for c in range(N_CORES)], axis=0).astype(np.int32)
    wts = np.concatenate([results[c]["out_w"] for c in range(N_CORES)], axis=0)
    return idx, wts, logits


make_in_maps_active = None  # set below


def kernel(hidden_states, weight, e_score_correction_bias):
    nc = _get_nc()
    in_maps = make_in_maps_active(hidden_states, weight, e_score_correction_bias)
    res = run_bass_kernel_spmd(nc, in_maps, list(range(N_CORES)))
    return assemble(res.results)


make_in_maps_active = make_in_maps_f16


# revision 22
# speedup vs baseline: 1.0457x; 1.0457x over previous
"""DeepSeek-V3 TopK router kernel for 8 Trainium2 NeuronCores.

Strategy (data/sequence parallel per sharding hint):
 - Shard the 16384 tokens across 8 cores (2048 tokens each); replicate the
   router weight + bias.
 - Host-side layout prep: x and w are transposed so the contraction dim
   (hidden) lands on SBUF partitions, scaled by exact powers of two
   (x*2^10, w*2^13) and split into fp16 hi+lo halves. Total DMA bytes are
   unchanged vs fp32 (2+2 bytes/elem) and the per-product error (~2^-21)
   sits below fp32 summation noise, while the PE runs at full 16-bit rate
   (3 passes) instead of native-fp32's 4x cost.
 - Device per core (build_nc_f16): per 128-token tile, 168 accumulating
   fp16 matmuls (lhsT = x^T chunk [128h,128t] stationary, rhs = w^T chunk
   [128h,256e] moving, fp32 PSUM [128t,256e]); hidden dim is DMA'd in
   7 chunks of 8x128 so matmuls start before whole tensors land; sigmoid
   (with 2^-23 descale) on ScalarE; group top-2 / top-4 and masked top-8
   via the DVE max8/max_index ops; score gather by position (iota ==
   index one-hot, fused scalar_tensor_tensor with accumulate — value
   matching would break on bit-equal score ties); normalize with the DVE
   iterative-divide reciprocal.
 - build_nc/make_in_maps keep the plain-fp32 variant (4x PE cost, same
   results) for reference.
"""

import sys

for _p in ("/opt/trn_rl_repo", "/root/.axon_site/_ro/trn_rl_repo"):
    if _p not in sys.path:
        sys.path.append(_p)

import numpy as np

import concourse.bass as bass
import concourse.bacc as bacc
import concourse.mybir as mybir
from concourse.bass import ds
from concourse.tile import TileContext
from concourse.bass_utils import run_bass_kernel_spmd

# Problem constants (hardcoded per contract)
T = 16384          # batch*seq = 4*4096
H = 7168           # hidden
E = 256            # experts
N_CORES = 8
TC = T // N_CORES  # tokens per core = 2048
KH = H // 128      # 56 contraction chunks
NG = 8             # expert groups
GS = E // NG       # group size = 32
TOPK = 8
SCALE = 2.5
TGROUP = 256       # tokens per DMA group (2 matmul subtiles)
CK = 8             # contraction chunks per DMA (k-chunking for overlap)
NCH = KH // CK     # 7 DMA chunks over the hidden dim

F32 = mybir.dt.float32
F32R = mybir.dt.float32r
U32 = mybir.dt.uint32


def build_nc(tc_tokens: int = TC, tgroup: int = TGROUP, debug: bool = False) -> bass.Bass:
    nc = bacc.Bacc(trn_type="TRN2")
    xt = nc.declare_dram_parameter("xt", [H, tc_tokens], F32, isOutput=False)
    wt = nc.declare_dram_parameter("wt", [H, E], F32, isOutput=False)
    biasb = nc.declare_dram_parameter("biasb", [128, E], F32, isOutput=False)
    iotab = nc.declare_dram_parameter("iotab", [128, E], F32, isOutput=False)
    out_logits = nc.declare_dram_parameter("out_logits", [tc_tokens, E], F32, isOutput=True)
    out_idx = nc.declare_dram_parameter("out_idx", [tc_tokens, TOPK], U32, isOutput=True)
    out_w = nc.declare_dram_parameter("out_w", [tc_tokens, TOPK], F32, isOutput=True)
    if debug:
        out_w8 = nc.declare_dram_parameter("out_w8", [tc_tokens, TOPK], F32, isOutput=True)
        out_den = nc.declare_dram_parameter("out_den", [tc_tokens, 2], F32, isOutput=True)
        out_eqs = nc.declare_dram_parameter("out_eqs", [tc_tokens, E], F32, isOutput=True)

    n_groups = tc_tokens // tgroup
    subs = tgroup // 128

    with TileContext(nc) as tc:
        with (
            tc.tile_pool(name="const", bufs=1) as cpool,
            tc.tile_pool(name="xs", bufs=2) as xpool,
            tc.tile_pool(name="ps", bufs=4, space="PSUM") as ppool,
            tc.tile_pool(name="work", bufs=2) as spool,
        ):
            wt_sb = cpool.tile([128, KH, E], F32)
            nc.sync.dma_start(out=wt_sb, in_=wt[:, :].rearrange("(k p) e -> p k e", p=128))
            bias_sb = cpool.tile([128, E], F32)
            nc.sync.dma_start(out=bias_sb, in_=biasb[:, :])
            iota_sb = cpool.tile([128, E], F32)
            nc.sync.dma_start(out=iota_sb, in_=iotab[:, :])
            # Dummy ldweights consuming wt_sb: folds the weight-DMA wait into
            # the PE's clock so the first real matmul carries only the x-tile
            # wait (the LDWEIGHTS ISA slot fits a single sync wait). The real
            # matmuls self-load their stationary operands, overwriting this.
            nc.tensor.ldweights(wt_sb[:, 0, 0:64].bitcast(mybir.dt.bfloat16))

            for g in range(n_groups):
                xg = xpool.tile([128, KH, tgroup], F32, tag="xg")
                nc.sync.dma_start(
                    out=xg,
                    in_=xt[:, ds(g * tgroup, tgroup)].rearrange("(k p) t -> p k t", p=128),
                )
                for sub in range(subs):
                    t0 = g * tgroup + sub * 128
                    lg = ppool.tile([128, E], F32, tag="lg")
                    for k in range(KH):
                        nc.tensor.matmul(
                            lg,
                            xg[:, k, ds(sub * 128, 128)],
                            wt_sb[:, k, :],
                            start=(k == 0),
                            stop=(k == KH - 1),
                        )
                    # router logits: PSUM -> SBUF -> DRAM
                    lsb = spool.tile([128, E], F32, tag="lsb")
                    nc.scalar.copy(lsb, lg)
                    nc.scalar.dma_start(out=out_logits[ds(t0, 128), :], in_=lsb)
                    scores = spool.tile([128, E], F32, tag="scores")
                    nc.scalar.activation(scores, lg, mybir.ActivationFunctionType.Sigmoid)
                    s4c = spool.tile([128, E], F32, tag="s4c")
                    nc.vector.tensor_add(s4c, scores, bias_sb)

                    # per-group top-2 -> group score
                    gtop = spool.tile([128, NG, 8], F32, tag="gtop")
                    for j in range(NG):
                        nc.vector.max(out=gtop[:, j, :], in_=s4c[:, ds(j * GS, GS)])
                    gsum = spool.tile([128, NG], F32, tag="gsum")
                    nc.vector.tensor_add(gsum, gtop[:, :, 0], gtop[:, :, 1])
                    # top-4 groups -> mask
                    gs8 = spool.tile([128, 8], F32, tag="gs8")
                    nc.vector.max(out=gs8, in_=gsum)
                    gmask = spool.tile([128, NG], F32, tag="gmask")
                    nc.vector.tensor_scalar(
                        gmask, gsum, gs8[:, 3:4], None, op0=mybir.AluOpType.is_ge
                    )
                    masked = spool.tile([128, E], F32, tag="masked")
                    for j in range(NG):
                        nc.vector.tensor_scalar_mul(
                            masked[:, ds(j * GS, GS)], s4c[:, ds(j * GS, GS)], gmask[:, j : j + 1]
                        )
                    # masked top-8 with indices
                    top8v = spool.tile([128, 8], F32, tag="top8v")
                    nc.vector.max(out=top8v, in_=masked)
                    idx8 = spool.tile([128, 8], U32, tag="idx8")
                    nc.vector.max_index(idx8, top8v, masked)
                    # gather scores at the top-8 positions by POSITION
                    # (iota == idx_k is a guaranteed one-hot; value-matching
                    # breaks when two experts tie bit-exactly)
                    idx8f = spool.tile([128, TOPK], F32, tag="idx8f")
                    nc.vector.tensor_copy(idx8f, idx8)
                    w8 = spool.tile([128, TOPK], F32, tag="w8")
                    eqs = spool.tile([128, E], F32, tag="eqs")
                    for k in range(TOPK):
                        nc.vector.scalar_tensor_tensor(
                            out=eqs,
                            in0=iota_sb,
                            scalar=idx8f[:, k : k + 1],
                            in1=scores,
                            op0=mybir.AluOpType.is_equal,
                            op1=mybir.AluOpType.mult,
                            accum_out=w8[:, k : k + 1],
                        )
                    denom = spool.tile([128, 1], F32, tag="denom")
                    nc.vector.reduce_sum(denom, w8, axis=mybir.AxisListType.X)
                    rden = spool.tile([128, 1], F32, tag="rden")
                    nc.vector.reciprocal(rden, denom)
                    wout = spool.tile([128, TOPK], F32, tag="wout")
                    nc.vector.tensor_scalar(
                        wout, w8, rden, SCALE,
                        op0=mybir.AluOpType.mult, op1=mybir.AluOpType.mult,
                    )
                    nc.scalar.dma_start(out=out_idx[ds(t0, 128), :], in_=idx8)
                    nc.scalar.dma_start(out=out_w[ds(t0, 128), :], in_=wout)
                    if debug:
                        nc.scalar.dma_start(out=out_w8[ds(t0, 128), :], in_=w8)
                        nc.scalar.dma_start(out=out_den[ds(t0, 128), 0:1], in_=denom)
                        nc.scalar.dma_start(out=out_den[ds(t0, 128), 1:2], in_=rden)
                        nc.scalar.dma_start(out=out_eqs[ds(t0, 128), :], in_=eqs)
    nc.finalize()
    return nc


F16 = mybir.dt.float16
XSCALE = 1024.0       # 2**10: keeps x_lo out of fp16 denormals
WSCALE = 8192.0       # 2**13: keeps w_lo out of fp16 denormals
DESCALE = 1.0 / (XSCALE * WSCALE)  # 2**-23, exact power of two


def build_nc_f16(tc_tokens: int = TC, tgroup: int = TGROUP, debug: bool = False) -> bass.Bass:
    """fp16 hi/lo 3-pass matmul variant: x' = x*2^10 = xhi+xlo (fp16),
    w' = w*2^13 = whi+wlo (fp16); logits' = xhi*whi + xlo*whi + xhi*wlo
    accumulated in fp32 PSUM; descale by 2^-23 (exact) on the way out.
    Error ~2^-21 per product: below fp32 summation noise, at 3x bf16-rate
    PE cost instead of 4x for native fp32."""
    nc = bacc.Bacc(trn_type="TRN2")
    xhi = nc.declare_dram_parameter("xhi", [H, tc_tokens], F16, isOutput=False)
    xlo = nc.declare_dram_parameter("xlo", [H, tc_tokens], F16, isOutput=False)
    whi = nc.declare_dram_parameter("whi", [H, E], F16, isOutput=False)
    wlo = nc.declare_dram_parameter("wlo", [H, E], F16, isOutput=False)
    biasb = nc.declare_dram_parameter("biasb", [128, E], F32, isOutput=False)
    iotab = nc.declare_dram_parameter("iotab", [128, E], F32, isOutput=False)
    out_logits = nc.declare_dram_parameter("out_logits", [tc_tokens, E], F32, isOutput=True)
    out_idx = nc.declare_dram_parameter("out_idx", [tc_tokens, TOPK], U32, isOutput=True)
    out_w = nc.declare_dram_parameter("out_w", [tc_tokens, TOPK], F32, isOutput=True)

    n_groups = tc_tokens // tgroup
    subs = tgroup // 128

    with TileContext(nc) as tc:
        with (
            tc.tile_pool(name="const", bufs=1) as cpool,
            tc.tile_pool(name="xs", bufs=2) as xpool,
            tc.tile_pool(name="ps", bufs=4, space="PSUM") as ppool,
            tc.tile_pool(name="work", bufs=2) as spool,
        ):
            # K is processed in chunks so matmuls can start as soon as the
            # first slices of the weights and x land, instead of waiting for
            # whole-tensor DMAs (~15 MB) to finish.
            # Prologue: interleave weight-chunk and group-0 x-chunk DMAs so
            # the first matmul's pair (whi_c0 + xghi_c0) lands first; bias and
            # iota (needed only by the DVE stage) go last.
            # whi and wlo are packed side by side in one tile so a single
            # N=512 matmul covers both hi passes for a given xhi stationary
            # (halves summed after accumulation; saves 1/3 of the LDWEIGHTS).
            w_ch = []
            for c in range(NCH):
                w_ch.append(cpool.tile([128, CK, 2 * E], F16, tag=f"w{c}", name=f"w_ch{c}"))

            def load_w_chunk(c):
                nc.sync.dma_start(
                    out=w_ch[c][:, :, 0:E],
                    in_=whi[ds(c * CK * 128, CK * 128), :].rearrange("(k p) e -> p k e", p=128))
                nc.sync.dma_start(
                    out=w_ch[c][:, :, E : 2 * E],
                    in_=wlo[ds(c * CK * 128, CK * 128), :].rearrange("(k p) e -> p k e", p=128))

            def load_x_chunk(g, c):
                xh = xpool.tile([128, CK, tgroup], F16, tag=f"xghi{c}")
                nc.sync.dma_start(
                    out=xh,
                    in_=xhi[ds(c * CK * 128, CK * 128), ds(g * tgroup, tgroup)]
                    .rearrange("(k p) t -> p k t", p=128))
                xl = xpool.tile([128, CK, tgroup], F16, tag=f"xglo{c}")
                nc.sync.dma_start(
                    out=xl,
                    in_=xlo[ds(c * CK * 128, CK * 128), ds(g * tgroup, tgroup)]
                    .rearrange("(k p) t -> p k t", p=128))
                return xh, xl

            # Weight chunks run 3 ahead of group-0 x chunks: the PE can
            # start after ~4 chunks (~11us) while the 3-chunk lead absorbs
            # DMA arrival jitter (starting with no lead stutters the PE and
            # re-throttles HAM; waiting for all weights idles it for 37us).
            W_LEAD = 2
            for c in range(W_LEAD):
                load_w_chunk(c)
            g0_chunks = []
            for c in range(NCH):
                g0_chunks.append(load_x_chunk(0, c))
                if c + W_LEAD < NCH:
                    load_w_chunk(c + W_LEAD)
            bias_sb = cpool.tile([128, E], F32)
            nc.sync.dma_start(out=bias_sb, in_=biasb[:, :])
            iota_sb = cpool.tile([128, E], F32)
            nc.sync.dma_start(out=iota_sb, in_=iotab[:, :])

            for g in range(n_groups):
                if g == 0:
                    xgpairs = g0_chunks
                else:
                    xgpairs = [load_x_chunk(g, c) for c in range(NCH)]
                xghi_ch = [p[0] for p in xgpairs]
                xglo_ch = [p[1] for p in xgpairs]
                for sub in range(subs):
                    t0 = g * tgroup + sub * 128
                    ts_ = ds(sub * 128, 128)
                    lg = ppool.tile([128, 2 * E], F32, tag="lg")
                    for k in range(KH):
                        c, kc = divmod(k, CK)
                        nc.tensor.matmul(lg, xghi_ch[c][:, kc, ts_], w_ch[c][:, kc, :],
                                         start=(k == 0), stop=False)
                        nc.tensor.matmul(lg[:, 0:E], xglo_ch[c][:, kc, ts_],
                                         w_ch[c][:, kc, 0:E],
                                         start=False, stop=(k == KH - 1))
                    # halves: lg[:,0:E] = xhi*whi + xlo*whi, lg[:,E:] = xhi*wlo
                    # (one PSUM operand per DVE op: stage the hi-lo half first)
                    lcor = spool.tile([128, E], F32, tag="lcor")
                    nc.scalar.copy(lcor, lg[:, E : 2 * E])
                    lsum = spool.tile([128, E], F32, tag="lsum")
                    nc.vector.tensor_add(lsum, lg[:, 0:E], lcor)
                    # descaled router logits: SBUF -> DRAM
                    lsb = spool.tile([128, E], F32, tag="lsb")
                    nc.scalar.mul(lsb, lsum, DESCALE)
                    nc.scalar.dma_start(out=out_logits[ds(t0, 128), :], in_=lsb)
                    scores = spool.tile([128, E], F32, tag="scores")
                    nc.scalar.activation(scores, lsum, mybir.ActivationFunctionType.Sigmoid,
                                         scale=DESCALE)
                    s4c = spool.tile([128, E], F32, tag="s4c")
                    nc.vector.tensor_add(s4c, scores, bias_sb)

                    gtop = spool.tile([128, NG, 8], F32, tag="gtop")
                    for j in range(NG):
                        nc.vector.max(out=gtop[:, j, :], in_=s4c[:, ds(j * GS, GS)])
                    gsum = spool.tile([128, NG], F32, tag="gsum")
                    nc.vector.tensor_add(gsum, gtop[:, :, 0], gtop[:, :, 1])
                    gs8 = spool.tile([128, 8], F32, tag="gs8")
                    nc.vector.max(out=gs8, in_=gsum)
                    gmask = spool.tile([128, NG], F32, tag="gmask")
                    nc.vector.tensor_scalar(
                        gmask, gsum, gs8[:, 3:4], None, op0=mybir.AluOpType.is_ge
                    )
                    masked = spool.tile([128, E], F32, tag="masked")
                    for j in range(NG):
                        nc.vector.tensor_scalar_mul(
                            masked[:, ds(j * GS, GS)], s4c[:, ds(j * GS, GS)], gmask[:, j : j + 1]
                        )
                    top8v = spool.tile([128, 8], F32, tag="top8v")
                    nc.vector.max(out=top8v, in_=masked)
                    idx8 = spool.tile([128, 8], U32, tag="idx8")
                    nc.vector.max_index(idx8, top8v, masked)
                    idx8f = spool.tile([128, TOPK], F32, tag="idx8f")
                    nc.vector.tensor_copy(idx8f, idx8)
                    w8 = spool.tile([128, TOPK], F32, tag="w8")
                    eqs = spool.tile([128, E], F32, tag="eqs")
                    for k in range(TOPK):
                        nc.vector.scalar_tensor_tensor(
                            out=eqs,
                            in0=iota_sb,
                            scalar=idx8f[:, k : k + 1],
                            in1=scores,
                            op0=mybir.AluOpType.is_equal,
                            op1=mybir.AluOpType.mult,
                            accum_out=w8[:, k : k + 1],
                        )
                    denom = spool.tile([128, 1], F32, tag="denom")
                    nc.vector.reduce_sum(denom, w8, axis=mybir.AxisListType.X)
                    rden = spool.tile([128, 1], F32, tag="rden")
                    nc.vector.reciprocal(rden, denom)
                    wout = spool.tile([128, TOPK], F32, tag="wout")
                    nc.vector.tensor_scalar(
                        wout, w8, rden, SCALE,
                        op0=mybir.AluOpType.mult, op1=mybir.AluOpType.mult,
                    )
                    nc.scalar.dma_start(out=out_idx[ds(t0, 128), :], in_=idx8)
                    nc.scalar.dma_start(out=out_w[ds(t0, 128), :], in_=wout)
    nc.finalize()
    return nc


def make_in_maps_f16(hidden_states, weight, e_score_correction_bias):
    from concurrent.futures import ThreadPoolExecutor

    x = np.ascontiguousarray(np.asarray(hidden_states, dtype=np.float32)).reshape(T, H)
    w = np.asarray(weight, dtype=np.float32)
    b = np.asarray(e_score_correction_bias, dtype=np.float32)
    ws = np.ascontiguousarray(w.T) * np.float32(WSCALE)
    whi = ws.astype(np.float16)
    wlo = (ws - whi.astype(np.float32)).astype(np.float16)
    biasb = np.ascontiguousarray(np.broadcast_to(b[None, :], (128, E)))
    iotab = np.ascontiguousarray(
        np.broadcast_to(np.arange(E, dtype=np.float32)[None, :], (128, E)))
    xt_full = x.T  # view

    def prep(c):
        xs = np.ascontiguousarray(xt_full[:, c * TC : (c + 1) * TC]) * np.float32(XSCALE)
        xhi = xs.astype(np.float16)
        xlo = (xs - xhi.astype(np.float32)).astype(np.float16)
        return {"xhi": xhi, "xlo": xlo, "whi": whi, "wlo": wlo,
                "biasb": biasb, "iotab": iotab}

    with ThreadPoolExecutor(N_CORES) as ex:
        return list(ex.map(prep, range(N_CORES)))


_NC = None


def _get_nc():
    global _NC
    if _NC is None:
        _NC = build_nc_f16()
    return _NC


def make_in_maps(hidden_states, weight, e_score_correction_bias):
    x = np.ascontiguousarray(np.asarray(hidden_states, dtype=np.float32)).reshape(T, H)
    w = np.asarray(weight, dtype=np.float32)
    b = np.asarray(e_score_correction_bias, dtype=np.float32)
    wt = np.ascontiguousarray(w.T)
    biasb = np.ascontiguousarray(np.broadcast_to(b[None, :], (128, E)))
    iotab = np.ascontiguousarray(np.broadcast_to(np.arange(E, dtype=np.float32)[None, :], (128, E)))
    xt_full = x.T  # view
    in_maps = []
    for c in range(N_CORES):
        xt_c = np.ascontiguousarray(xt_full[:, c * TC : (c + 1) * TC])
        in_maps.append({"xt": xt_c, "wt": wt, "biasb": biasb, "iotab": iotab})
    return in_maps


def assemble(results):
    logits = np.concatenate([results[c]["out_logits"] for c in range(N_CORES)], axis=0)
    idx = np.concatenate([results[c]["out_idx"] for c in range(N_CORES)], axis=0).astype(np.int32)
    wts = np.concatenate([results[c]["out_w"] for c in range(N_CORES)], axis=0)
    return idx, wts, logits


make_in_maps_active = None  # set below


def kernel(hidden_states, weight, e_score_correction_bias):
    nc = _get_nc()
    in_maps = make_in_maps_active(hidden_states, weight, e_score_correction_bias)
    res = run_bass_kernel_spmd(nc, in_maps, list(range(N_CORES)))
    return assemble(res.results)


make_in_maps_active = make_in_maps_f16


# revision 23
# speedup vs baseline: 1.0535x; 1.0075x over previous
"""DeepSeek-V3 TopK router kernel for 8 Trainium2 NeuronCores.

Strategy (data/sequence parallel per sharding hint):
 - Shard the 16384 tokens across 8 cores (2048 tokens each); replicate the
   router weight + bias.
 - Host-side layout prep: x and w are transposed so the contraction dim
   (hidden) lands on SBUF partitions, scaled by exact powers of two
   (x*2^10, w*2^13) and split into fp16 hi+lo halves. Total DMA bytes are
   unchanged vs fp32 (2+2 bytes/elem) and the per-product error (~2^-21)
   sits below fp32 summation noise, while the PE runs at full 16-bit rate
   (3 passes) instead of native-fp32's 4x cost.
 - Device per core (build_nc_f16): per 128-token tile, 168 accumulating
   fp16 matmuls (lhsT = x^T chunk [128h,128t] stationary, rhs = w^T chunk
   [128h,256e] moving, fp32 PSUM [128t,256e]); hidden dim is DMA'd in
   7 chunks of 8x128 so matmuls start before whole tensors land; sigmoid
   (with 2^-23 descale) on ScalarE; group top-2 / top-4 and masked top-8
   via the DVE max8/max_index ops; score gather by position (iota ==
   index one-hot, fused scalar_tensor_tensor with accumulate — value
   matching would break on bit-equal score ties); normalize with the DVE
   iterative-divide reciprocal.
 - build_nc/make_in_maps keep the plain-fp32 variant (4x PE cost, same
   results) for reference.
"""

import sys

for _p in ("/opt/trn_rl_repo", "/root/.axon_site/_ro/trn_rl_repo"):
    if _p not in sys.path:
        sys.path.append(_p)

import numpy as np

import concourse.bass as bass
import concourse.bacc as bacc
import concourse.mybir as mybir
from concourse.bass import ds
from concourse.tile import TileContext
from concourse.bass_utils import run_bass_kernel_spmd

# Problem constants (hardcoded per contract)
T = 16384          # batch*seq = 4*4096
H = 7168           # hidden
E = 256            # experts
N_CORES = 8
TC = T // N_CORES  # tokens per core = 2048
KH = H // 128      # 56 contraction chunks
NG = 8             # expert groups
GS = E // NG       # group size = 32
TOPK = 8
SCALE = 2.5
TGROUP = 256       # tokens per DMA group (2 matmul subtiles)
CK = 8             # contraction chunks per DMA (k-chunking for overlap)
NCH = KH // CK     # 7 DMA chunks over the hidden dim

F32 = mybir.dt.float32
F32R = mybir.dt.float32r
U32 = mybir.dt.uint32


def build_nc(tc_tokens: int = TC, tgroup: int = TGROUP, debug: bool = False) -> bass.Bass:
    nc = bacc.Bacc(trn_type="TRN2")
    xt = nc.declare_dram_parameter("xt", [H, tc_tokens], F32, isOutput=False)
    wt = nc.declare_dram_parameter("wt", [H, E], F32, isOutput=False)
    biasb = nc.declare_dram_parameter("biasb", [128, E], F32, isOutput=False)
    iotab = nc.declare_dram_parameter("iotab", [128, E], F32, isOutput=False)
    out_logits = nc.declare_dram_parameter("out_logits", [tc_tokens, E], F32, isOutput=True)
    out_idx = nc.declare_dram_parameter("out_idx", [tc_tokens, TOPK], U32, isOutput=True)
    out_w = nc.declare_dram_parameter("out_w", [tc_tokens, TOPK], F32, isOutput=True)
    if debug:
        out_w8 = nc.declare_dram_parameter("out_w8", [tc_tokens, TOPK], F32, isOutput=True)
        out_den = nc.declare_dram_parameter("out_den", [tc_tokens, 2], F32, isOutput=True)
        out_eqs = nc.declare_dram_parameter("out_eqs", [tc_tokens, E], F32, isOutput=True)

    n_groups = tc_tokens // tgroup
    subs = tgroup // 128

    with TileContext(nc) as tc:
        with (
            tc.tile_pool(name="const", bufs=1) as cpool,
            tc.tile_pool(name="xs", bufs=2) as xpool,
            tc.tile_pool(name="ps", bufs=4, space="PSUM") as ppool,
            tc.tile_pool(name="work", bufs=2) as spool,
        ):
            wt_sb = cpool.tile([128, KH, E], F32)
            nc.sync.dma_start(out=wt_sb, in_=wt[:, :].rearrange("(k p) e -> p k e", p=128))
            bias_sb = cpool.tile([128, E], F32)
            nc.sync.dma_start(out=bias_sb, in_=biasb[:, :])
            iota_sb = cpool.tile([128, E], F32)
            nc.sync.dma_start(out=iota_sb, in_=iotab[:, :])
            # Dummy ldweights consuming wt_sb: folds the weight-DMA wait into
            # the PE's clock so the first real matmul carries only the x-tile
            # wait (the LDWEIGHTS ISA slot fits a single sync wait). The real
            # matmuls self-load their stationary operands, overwriting this.
            nc.tensor.ldweights(wt_sb[:, 0, 0:64].bitcast(mybir.dt.bfloat16))

            for g in range(n_groups):
                xg = xpool.tile([128, KH, tgroup], F32, tag="xg")
                nc.sync.dma_start(
                    out=xg,
                    in_=xt[:, ds(g * tgroup, tgroup)].rearrange("(k p) t -> p k t", p=128),
                )
                for sub in range(subs):
                    t0 = g * tgroup + sub * 128
                    lg = ppool.tile([128, E], F32, tag="lg")
                    for k in range(KH):
                        nc.tensor.matmul(
                            lg,
                            xg[:, k, ds(sub * 128, 128)],
                            wt_sb[:, k, :],
                            start=(k == 0),
                            stop=(k == KH - 1),
                        )
                    # router logits: PSUM -> SBUF -> DRAM
                    lsb = spool.tile([128, E], F32, tag="lsb")
                    nc.scalar.copy(lsb, lg)
                    nc.scalar.dma_start(out=out_logits[ds(t0, 128), :], in_=lsb)
                    scores = spool.tile([128, E], F32, tag="scores")
                    nc.scalar.activation(scores, lg, mybir.ActivationFunctionType.Sigmoid)
                    s4c = spool.tile([128, E], F32, tag="s4c")
                    nc.vector.tensor_add(s4c, scores, bias_sb)

                    # per-group top-2 -> group score
                    gtop = spool.tile([128, NG, 8], F32, tag="gtop")
                    for j in range(NG):
                        nc.vector.max(out=gtop[:, j, :], in_=s4c[:, ds(j * GS, GS)])
                    gsum = spool.tile([128, NG], F32, tag="gsum")
                    nc.vector.tensor_add(gsum, gtop[:, :, 0], gtop[:, :, 1])
                    # top-4 groups -> mask
                    gs8 = spool.tile([128, 8], F32, tag="gs8")
                    nc.vector.max(out=gs8, in_=gsum)
                    gmask = spool.tile([128, NG], F32, tag="gmask")
                    nc.vector.tensor_scalar(
                        gmask, gsum, gs8[:, 3:4], None, op0=mybir.AluOpType.is_ge
                    )
                    masked = spool.tile([128, E], F32, tag="masked")
                    for j in range(NG):
                        nc.vector.tensor_scalar_mul(
                            masked[:, ds(j * GS, GS)], s4c[:, ds(j * GS, GS)], gmask[:, j : j + 1]
                        )
                    # masked top-8 with indices
                    top8v = spool.tile([128, 8], F32, tag="top8v")
                    nc.vector.max(out=top8v, in_=masked)
                    idx8 = spool.tile([128, 8], U32, tag="idx8")
                    nc.vector.max_index(idx8, top8v, masked)
                    # gather scores at the top-8 positions by POSITION
                    # (iota == idx_k is a guaranteed one-hot; value-matching
                    # breaks when two experts tie bit-exactly)
                    idx8f = spool.tile([128, TOPK], F32, tag="idx8f")
                    nc.vector.tensor_copy(idx8f, idx8)
                    w8 = spool.tile([128, TOPK], F32, tag="w8")
                    eqs = spool.tile([128, E], F32, tag="eqs")
                    for k in range(TOPK):
                        nc.vector.scalar_tensor_tensor(
                            out=eqs,
                            in0=iota_sb,
                            scalar=idx8f[:, k : k + 1],
                            in1=scores,
                            op0=mybir.AluOpType.is_equal,
                            op1=mybir.AluOpType.mult,
                            accum_out=w8[:, k : k + 1],
                        )
                    denom = spool.tile([128, 1], F32, tag="denom")
                    nc.vector.reduce_sum(denom, w8, axis=mybir.AxisListType.X)
                    rden = spool.tile([128, 1], F32, tag="rden")
                    nc.vector.reciprocal(rden, denom)
                    wout = spool.tile([128, TOPK], F32, tag="wout")
                    nc.vector.tensor_scalar(
                        wout, w8, rden, SCALE,
                        op0=mybir.AluOpType.mult, op1=mybir.AluOpType.mult,
                    )
                    nc.scalar.dma_start(out=out_idx[ds(t0, 128), :], in_=idx8)
                    nc.scalar.dma_start(out=out_w[ds(t0, 128), :], in_=wout)
                    if debug:
                        nc.scalar.dma_start(out=out_w8[ds(t0, 128), :], in_=w8)
                        nc.scalar.dma_start(out=out_den[ds(t0, 128), 0:1], in_=denom)
                        nc.scalar.dma_start(out=out_den[ds(t0, 128), 1:2], in_=rden)
                        nc.scalar.dma_start(out=out_eqs[ds(t0, 128), :], in_=eqs)
    nc.finalize()
    return nc


F16 = mybir.dt.float16
XSCALE = 1024.0       # 2**10: keeps x_lo out of fp16 denormals
WSCALE = 8192.0       # 2**13: keeps w_lo out of fp16 denormals
DESCALE = 1.0 / (XSCALE * WSCALE)  # 2**-23, exact power of two


def build_nc_f16(tc_tokens: int = TC, tgroup: int = TGROUP, debug: bool = False) -> bass.Bass:
    """fp16 hi/lo 3-pass matmul variant: x' = x*2^10 = xhi+xlo (fp16),
    w' = w*2^13 = whi+wlo (fp16); logits' = xhi*whi + xlo*whi + xhi*wlo
    accumulated in fp32 PSUM; descale by 2^-23 (exact) on the way out.
    Error ~2^-21 per product: below fp32 summation noise, at 3x bf16-rate
    PE cost instead of 4x for native fp32."""
    nc = bacc.Bacc(trn_type="TRN2")
    xhi = nc.declare_dram_parameter("xhi", [H, tc_tokens], F16, isOutput=False)
    xlo = nc.declare_dram_parameter("xlo", [H, tc_tokens], F16, isOutput=False)
    whi = nc.declare_dram_parameter("whi", [H, E], F16, isOutput=False)
    wlo = nc.declare_dram_parameter("wlo", [H, E], F16, isOutput=False)
    biasb = nc.declare_dram_parameter("biasb", [128, E], F32, isOutput=False)
    iotab = nc.declare_dram_parameter("iotab", [128, E], F32, isOutput=False)
    out_logits = nc.declare_dram_parameter("out_logits", [tc_tokens, E], F32, isOutput=True)
    out_idx = nc.declare_dram_parameter("out_idx", [tc_tokens, TOPK], U32, isOutput=True)
    out_w = nc.declare_dram_parameter("out_w", [tc_tokens, TOPK], F32, isOutput=True)

    n_groups = tc_tokens // tgroup
    subs = tgroup // 128

    with TileContext(nc) as tc:
        with (
            tc.tile_pool(name="const", bufs=1) as cpool,
            tc.tile_pool(name="xs", bufs=2) as xpool,
            tc.tile_pool(name="ps", bufs=4, space="PSUM") as ppool,
            tc.tile_pool(name="work", bufs=2) as spool,
        ):
            # K is processed in chunks so matmuls can start as soon as the
            # first slices of the weights and x land, instead of waiting for
            # whole-tensor DMAs (~15 MB) to finish.
            # Prologue: interleave weight-chunk and group-0 x-chunk DMAs so
            # the first matmul's pair (whi_c0 + xghi_c0) lands first; bias and
            # iota (needed only by the DVE stage) go last.
            # whi and wlo are packed side by side in one tile so a single
            # N=512 matmul covers both hi passes for a given xhi stationary
            # (halves summed after accumulation; saves 1/3 of the LDWEIGHTS).
            w_ch = []
            for c in range(NCH):
                w_ch.append(cpool.tile([128, CK, 2 * E], F16, tag=f"w{c}", name=f"w_ch{c}"))

            def load_w_chunk(c):
                nc.sync.dma_start(
                    out=w_ch[c][:, :, 0:E],
                    in_=whi[ds(c * CK * 128, CK * 128), :].rearrange("(k p) e -> p k e", p=128))
                nc.sync.dma_start(
                    out=w_ch[c][:, :, E : 2 * E],
                    in_=wlo[ds(c * CK * 128, CK * 128), :].rearrange("(k p) e -> p k e", p=128))

            def load_x_chunk(g, c):
                xh = xpool.tile([128, CK, tgroup], F16, tag=f"xghi{c}")
                nc.sync.dma_start(
                    out=xh,
                    in_=xhi[ds(c * CK * 128, CK * 128), ds(g * tgroup, tgroup)]
                    .rearrange("(k p) t -> p k t", p=128))
                xl = xpool.tile([128, CK, tgroup], F16, tag=f"xglo{c}")
                nc.sync.dma_start(
                    out=xl,
                    in_=xlo[ds(c * CK * 128, CK * 128), ds(g * tgroup, tgroup)]
                    .rearrange("(k p) t -> p k t", p=128))
                return xh, xl

            # Weight chunks run 3 ahead of group-0 x chunks: the PE can
            # start after ~4 chunks (~11us) while the 3-chunk lead absorbs
            # DMA arrival jitter (starting with no lead stutters the PE and
            # re-throttles HAM; waiting for all weights idles it for 37us).
            W_LEAD = 3
            for c in range(W_LEAD):
                load_w_chunk(c)
            g0_chunks = []
            for c in range(NCH):
                g0_chunks.append(load_x_chunk(0, c))
                if c + W_LEAD < NCH:
                    load_w_chunk(c + W_LEAD)
            bias_sb = cpool.tile([128, E], F32)
            nc.sync.dma_start(out=bias_sb, in_=biasb[:, :])
            iota_sb = cpool.tile([128, E], F32)
            nc.sync.dma_start(out=iota_sb, in_=iotab[:, :])

            for g in range(n_groups):
                if g == 0:
                    xgpairs = g0_chunks
                else:
                    xgpairs = [load_x_chunk(g, c) for c in range(NCH)]
                xghi_ch = [p[0] for p in xgpairs]
                xglo_ch = [p[1] for p in xgpairs]
                for sub in range(subs):
                    t0 = g * tgroup + sub * 128
                    ts_ = ds(sub * 128, 128)
                    lg = ppool.tile([128, 2 * E], F32, tag="lg")
                    for k in range(KH):
                        c, kc = divmod(k, CK)
                        nc.tensor.matmul(lg, xghi_ch[c][:, kc, ts_], w_ch[c][:, kc, :],
                                         start=(k == 0), stop=False)
                        nc.tensor.matmul(lg[:, 0:E], xglo_ch[c][:, kc, ts_],
                                         w_ch[c][:, kc, 0:E],
                                         start=False, stop=(k == KH - 1))
                    # halves: lg[:,0:E] = xhi*whi + xlo*whi, lg[:,E:] = xhi*wlo
                    # (one PSUM operand per DVE op: stage the hi-lo half first)
                    lcor = spool.tile([128, E], F32, tag="lcor")
                    nc.scalar.copy(lcor, lg[:, E : 2 * E])
                    lsum = spool.tile([128, E], F32, tag="lsum")
                    nc.vector.tensor_add(lsum, lg[:, 0:E], lcor)
                    # descaled router logits: SBUF -> DRAM
                    lsb = spool.tile([128, E], F32, tag="lsb")
                    nc.scalar.mul(lsb, lsum, DESCALE)
                    nc.scalar.dma_start(out=out_logits[ds(t0, 128), :], in_=lsb)
                    scores = spool.tile([128, E], F32, tag="scores")
                    nc.scalar.activation(scores, lsum, mybir.ActivationFunctionType.Sigmoid,
                                         scale=DESCALE)
                    s4c = spool.tile([128, E], F32, tag="s4c")
                    nc.vector.tensor_add(s4c, scores, bias_sb)

                    gtop = spool.tile([128, NG, 8], F32, tag="gtop")
                    for j in range(NG):
                        nc.vector.max(out=gtop[:, j, :], in_=s4c[:, ds(j * GS, GS)])
                    gsum = spool.tile([128, NG], F32, tag="gsum")
                    nc.vector.tensor_add(gsum, gtop[:, :, 0], gtop[:, :, 1])
                    gs8 = spool.tile([128, 8], F32, tag="gs8")
                    nc.vector.max(out=gs8, in_=gsum)
                    gmask = spool.tile([128, NG], F32, tag="gmask")
                    nc.vector.tensor_scalar(
                        gmask, gsum, gs8[:, 3:4], None, op0=mybir.AluOpType.is_ge
                    )
                    masked = spool.tile([128, E], F32, tag="masked")
                    for j in range(NG):
                        nc.vector.tensor_scalar_mul(
                            masked[:, ds(j * GS, GS)], s4c[:, ds(j * GS, GS)], gmask[:, j : j + 1]
                        )
                    top8v = spool.tile([128, 8], F32, tag="top8v")
                    nc.vector.max(out=top8v, in_=masked)
                    idx8 = spool.tile([128, 8], U32, tag="idx8")
                    nc.vector.max_index(idx8, top8v, masked)
                    idx8f = spool.tile([128, TOPK], F32, tag="idx8f")
                    nc.vector.tensor_copy(idx8f, idx8)
                    w8 = spool.tile([128, TOPK], F32, tag="w8")
                    eqs = spool.tile([128, E], F32, tag="eqs")
                    for k in range(TOPK):
                        nc.vector.scalar_tensor_tensor(
                            out=eqs,
                            in0=iota_sb,
                            scalar=idx8f[:, k : k + 1],
                            in1=scores,
                            op0=mybir.AluOpType.is_equal,
                            op1=mybir.AluOpType.mult,
                            accum_out=w8[:, k : k + 1],
                        )
                    denom = spool.tile([128, 1], F32, tag="denom")
                    nc.vector.reduce_sum(denom, w8, axis=mybir.AxisListType.X)
                    rden = spool.tile([128, 1], F32, tag="rden")
                    nc.vector.reciprocal(rden, denom)
                    wout = spool.tile([128, TOPK], F32, tag="wout")
                    nc.vector.tensor_scalar(
                        wout, w8, rden, SCALE,
                        op0=mybir.AluOpType.mult, op1=mybir.AluOpType.mult,
                    )
                    nc.scalar.dma_start(out=out_idx[ds(t0, 128), :], in_=idx8)
                    nc.scalar.dma_start(out=out_w[ds(t0, 128), :], in_=wout)
    nc.finalize()
    return nc


def make_in_maps_f16(hidden_states, weight, e_score_correction_bias):
    from concurrent.futures import ThreadPoolExecutor

    x = np.ascontiguousarray(np.asarray(hidden_states, dtype=np.float32)).reshape(T, H)
    w = np.asarray(weight, dtype=np.float32)
    b = np.asarray(e_score_correction_bias, dtype=np.float32)
    ws = np.ascontiguousarray(w.T) * np.float32(WSCALE)
    whi = ws.astype(np.float16)
    wlo = (ws - whi.astype(np.float32)).astype(np.float16)
    biasb = np.ascontiguousarray(np.broadcast_to(b[None, :], (128, E)))
    iotab = np.ascontiguousarray(
        np.broadcast_to(np.arange(E, dtype=np.float32)[None, :], (128, E)))
    xt_full = x.T  # view

    def prep(c):
        xs = np.ascontiguousarray(xt_full[:, c * TC : (c + 1) * TC]) * np.float32(XSCALE)
        xhi = xs.astype(np.float16)
        xlo = (xs - xhi.astype(np.float32)).astype(np.float16)
        return {"xhi": xhi, "xlo": xlo, "whi": whi, "wlo": wlo,
                "biasb": biasb, "iotab": iotab}

    with ThreadPoolExecutor(N_CORES) as ex:
        return list(ex.map(prep, range(N_CORES)))


_NC = None


def _get_nc():
    global _NC
    if _NC is None:
        _NC = build_nc_f16()
    return _NC


def make_in_maps(hidden_states, weight, e_score_correction_bias):
    x = np.ascontiguousarray(np.asarray(hidden_states, dtype=np.float32)).reshape(T, H)
    w = np.asarray(weight, dtype=np.float32)
    b = np.asarray(e_score_correction_bias, dtype=np.float32)
    wt = np.ascontiguousarray(w.T)
    biasb = np.ascontiguousarray(np.broadcast_to(b[None, :], (128, E)))
    iotab = np.ascontiguousarray(np.broadcast_to(np.arange(E, dtype=np.float32)[None, :], (128, E)))
    xt_full = x.T  # view
    in_maps = []
    for c in range(N_CORES):
        xt_c = np.ascontiguousarray(xt_full[:, c * TC : (c + 1) * TC])
        in_maps.append({"xt": xt_c, "wt": wt, "biasb": biasb, "iotab": iotab})
    return in_maps


def assemble(results):
    logits = np.concatenate([results[c]["out_logits"] for c in range(N_CORES)], axis=0)
    idx = np.concatenate([results[c]["out_idx"] for c in range(N_CORES)], axis=0).astype(np.int32)
    wts = np.concatenate([results[c]["out_w"] for c in range(N_CORES)], axis=0)
    return idx, wts, logits


make_in_maps_active = None  # set below


def kernel(hidden_states, weight, e_score_correction_bias):
    nc = _get_nc()
    in_maps = make_in_maps_active(hidden_states, weight, e_score_correction_bias)
    res = run_bass_kernel_spmd(nc, in_maps, list(range(N_CORES)))
    return assemble(res.results)


make_in_maps_active = make_in_maps_f16


# revision 24
# speedup vs baseline: 1.0543x; 1.0008x over previous
"""DeepSeek-V3 TopK router kernel for 8 Trainium2 NeuronCores.

Strategy (data/sequence parallel per sharding hint):
 - Shard the 16384 tokens across 8 cores (2048 tokens each); replicate the
   router weight + bias.
 - Host-side layout prep: x and w are transposed so the contraction dim
   (hidden) lands on SBUF partitions, scaled by exact powers of two
   (x*2^10, w*2^13) and split into fp16 hi+lo halves. Total DMA bytes are
   unchanged vs fp32 (2+2 bytes/elem) and the per-product error (~2^-21)
   sits below fp32 summation noise, while the PE runs at full 16-bit rate
   (3 passes) instead of native-fp32's 4x cost.
 - Device per core (build_nc_f16): per 128-token tile, 168 accumulating
   fp16 matmuls (lhsT = x^T chunk [128h,128t] stationary, rhs = w^T chunk
   [128h,256e] moving, fp32 PSUM [128t,256e]); hidden dim is DMA'd in
   7 chunks of 8x128 so matmuls start before whole tensors land; sigmoid
   (with 2^-23 descale) on ScalarE; group top-2 / top-4 and masked top-8
   via the DVE max8/max_index ops; score gather by position (iota ==
   index one-hot, fused scalar_tensor_tensor with accumulate — value
   matching would break on bit-equal score ties); normalize with the DVE
   iterative-divide reciprocal.
 - build_nc/make_in_maps keep the plain-fp32 variant (4x PE cost, same
   results) for reference.
"""

import sys

for _p in ("/opt/trn_rl_repo", "/root/.axon_site/_ro/trn_rl_repo"):
    if _p not in sys.path:
        sys.path.append(_p)

import numpy as np

import concourse.bass as bass
import concourse.bacc as bacc
import concourse.mybir as mybir
from concourse.bass import ds
from concourse.tile import TileContext
from concourse.bass_utils import run_bass_kernel_spmd

# Problem constants (hardcoded per contract)
T = 16384          # batch*seq = 4*4096
H = 7168           # hidden
E = 256            # experts
N_CORES = 8
TC = T // N_CORES  # tokens per core = 2048
KH = H // 128      # 56 contraction chunks
NG = 8             # expert groups
GS = E // NG       # group size = 32
TOPK = 8
SCALE = 2.5
TGROUP = 256       # tokens per DMA group (2 matmul subtiles)
CK = 8             # contraction chunks per DMA (k-chunking for overlap)
NCH = KH // CK     # 7 DMA chunks over the hidden dim

F32 = mybir.dt.float32
F32R = mybir.dt.float32r
U32 = mybir.dt.uint32


def build_nc(tc_tokens: int = TC, tgroup: int = TGROUP, debug: bool = False) -> bass.Bass:
    nc = bacc.Bacc(trn_type="TRN2")
    xt = nc.declare_dram_parameter("xt", [H, tc_tokens], F32, isOutput=False)
    wt = nc.declare_dram_parameter("wt", [H, E], F32, isOutput=False)
    biasb = nc.declare_dram_parameter("biasb", [128, E], F32, isOutput=False)
    iotab = nc.declare_dram_parameter("iotab", [128, E], F32, isOutput=False)
    out_logits = nc.declare_dram_parameter("out_logits", [tc_tokens, E], F32, isOutput=True)
    out_idx = nc.declare_dram_parameter("out_idx", [tc_tokens, TOPK], U32, isOutput=True)
    out_w = nc.declare_dram_parameter("out_w", [tc_tokens, TOPK], F32, isOutput=True)
    if debug:
        out_w8 = nc.declare_dram_parameter("out_w8", [tc_tokens, TOPK], F32, isOutput=True)
        out_den = nc.declare_dram_parameter("out_den", [tc_tokens, 2], F32, isOutput=True)
        out_eqs = nc.declare_dram_parameter("out_eqs", [tc_tokens, E], F32, isOutput=True)

    n_groups = tc_tokens // tgroup
    subs = tgroup // 128

    with TileContext(nc) as tc:
        with (
            tc.tile_pool(name="const", bufs=1) as cpool,
            tc.tile_pool(name="xs", bufs=2) as xpool,
            tc.tile_pool(name="ps", bufs=5, space="PSUM") as ppool,
            tc.tile_pool(name="work", bufs=3) as spool,
        ):
            wt_sb = cpool.tile([128, KH, E], F32)
            nc.sync.dma_start(out=wt_sb, in_=wt[:, :].rearrange("(k p) e -> p k e", p=128))
            bias_sb = cpool.tile([128, E], F32)
            nc.sync.dma_start(out=bias_sb, in_=biasb[:, :])
            iota_sb = cpool.tile([128, E], F32)
            nc.sync.dma_start(out=iota_sb, in_=iotab[:, :])
            # Dummy ldweights consuming wt_sb: folds the weight-DMA wait into
            # the PE's clock so the first real matmul carries only the x-tile
            # wait (the LDWEIGHTS ISA slot fits a single sync wait). The real
            # matmuls self-load their stationary operands, overwriting this.
            nc.tensor.ldweights(wt_sb[:, 0, 0:64].bitcast(mybir.dt.bfloat16))

            for g in range(n_groups):
                xg = xpool.tile([128, KH, tgroup], F32, tag="xg")
                nc.sync.dma_start(
                    out=xg,
                    in_=xt[:, ds(g * tgroup, tgroup)].rearrange("(k p) t -> p k t", p=128),
                )
                for sub in range(subs):
                    t0 = g * tgroup + sub * 128
                    lg = ppool.tile([128, E], F32, tag="lg")
                    for k in range(KH):
                        nc.tensor.matmul(
                            lg,
                            xg[:, k, ds(sub * 128, 128)],
                            wt_sb[:, k, :],
                            start=(k == 0),
                            stop=(k == KH - 1),
                        )
                    # router logits: PSUM -> SBUF -> DRAM
                    lsb = spool.tile([128, E], F32, tag="lsb")
                    nc.scalar.copy(lsb, lg)
                    nc.scalar.dma_start(out=out_logits[ds(t0, 128), :], in_=lsb)
                    scores = spool.tile([128, E], F32, tag="scores")
                    nc.scalar.activation(scores, lg, mybir.ActivationFunctionType.Sigmoid)
                    s4c = spool.tile([128, E], F32, tag="s4c")
                    nc.vector.tensor_add(s4c, scores, bias_sb)

                    # per-group top-2 -> group score
                    gtop = spool.tile([128, NG, 8], F32, tag="gtop")
                    for j in range(NG):
                        nc.vector.max(out=gtop[:, j, :], in_=s4c[:, ds(j * GS, GS)])
                    gsum = spool.tile([128, NG], F32, tag="gsum")
                    nc.vector.tensor_add(gsum, gtop[:, :, 0], gtop[:, :, 1])
                    # top-4 groups -> mask
                    gs8 = spool.tile([128, 8], F32, tag="gs8")
                    nc.vector.max(out=gs8, in_=gsum)
                    gmask = spool.tile([128, NG], F32, tag="gmask")
                    nc.vector.tensor_scalar(
                        gmask, gsum, gs8[:, 3:4], None, op0=mybir.AluOpType.is_ge
                    )
                    masked = spool.tile([128, E], F32, tag="masked")
                    nc.vector.tensor_tensor(
                        out=masked.rearrange("p (g s) -> p g s", g=NG),
                        in0=s4c.rearrange("p (g s) -> p g s", g=NG),
                        in1=gmask[:, :, None].to_broadcast([128, NG, GS]),
                        op=mybir.AluOpType.mult,
                    )
                    # masked top-8 with indices
                    top8v = spool.tile([128, 8], F32, tag="top8v")
                    nc.vector.max(out=top8v, in_=masked)
                    idx8 = spool.tile([128, 8], U32, tag="idx8")
                    nc.vector.max_index(idx8, top8v, masked)
                    # gather scores at the top-8 positions by POSITION
                    # (iota == idx_k is a guaranteed one-hot; value-matching
                    # breaks when two experts tie bit-exactly)
                    idx8f = spool.tile([128, TOPK], F32, tag="idx8f")
                    nc.vector.tensor_copy(idx8f, idx8)
                    w8 = spool.tile([128, TOPK], F32, tag="w8")
                    eqs = spool.tile([128, E], F32, tag="eqs")
                    for k in range(TOPK):
                        nc.vector.scalar_tensor_tensor(
                            out=eqs,
                            in0=iota_sb,
                            scalar=idx8f[:, k : k + 1],
                            in1=scores,
                            op0=mybir.AluOpType.is_equal,
                            op1=mybir.AluOpType.mult,
                            accum_out=w8[:, k : k + 1],
                        )
                    denom = spool.tile([128, 1], F32, tag="denom")
                    nc.vector.reduce_sum(denom, w8, axis=mybir.AxisListType.X)
                    rden = spool.tile([128, 1], F32, tag="rden")
                    nc.vector.reciprocal(rden, denom)
                    wout = spool.tile([128, TOPK], F32, tag="wout")
                    nc.vector.tensor_scalar(
                        wout, w8, rden, SCALE,
                        op0=mybir.AluOpType.mult, op1=mybir.AluOpType.mult,
                    )
                    nc.scalar.dma_start(out=out_idx[ds(t0, 128), :], in_=idx8)
                    nc.scalar.dma_start(out=out_w[ds(t0, 128), :], in_=wout)
                    if debug:
                        nc.scalar.dma_start(out=out_w8[ds(t0, 128), :], in_=w8)
                        nc.scalar.dma_start(out=out_den[ds(t0, 128), 0:1], in_=denom)
                        nc.scalar.dma_start(out=out_den[ds(t0, 128), 1:2], in_=rden)
                        nc.scalar.dma_start(out=out_eqs[ds(t0, 128), :], in_=eqs)
    nc.finalize()
    return nc


F16 = mybir.dt.float16
XSCALE = 1024.0       # 2**10: keeps x_lo out of fp16 denormals
WSCALE = 8192.0       # 2**13: keeps w_lo out of fp16 denormals
DESCALE = 1.0 / (XSCALE * WSCALE)  # 2**-23, exact power of two


def build_nc_f16(tc_tokens: int = TC, tgroup: int = TGROUP, debug: bool = False) -> bass.Bass:
    """fp16 hi/lo 3-pass matmul variant: x' = x*2^10 = xhi+xlo (fp16),
    w' = w*2^13 = whi+wlo (fp16); logits' = xhi*whi + xlo*whi + xhi*wlo
    accumulated in fp32 PSUM; descale by 2^-23 (exact) on the way out.
    Error ~2^-21 per product: below fp32 summation noise, at 3x bf16-rate
    PE cost instead of 4x for native fp32."""
    nc = bacc.Bacc(trn_type="TRN2")
    xhi = nc.declare_dram_parameter("xhi", [H, tc_tokens], F16, isOutput=False)
    xlo = nc.declare_dram_parameter("xlo", [H, tc_tokens], F16, isOutput=False)
    whi = nc.declare_dram_parameter("whi", [H, E], F16, isOutput=False)
    wlo = nc.declare_dram_parameter("wlo", [H, E], F16, isOutput=False)
    biasb = nc.declare_dram_parameter("biasb", [128, E], F32, isOutput=False)
    iotab = nc.declare_dram_parameter("iotab", [128, E], F32, isOutput=False)
    out_logits = nc.declare_dram_parameter("out_logits", [tc_tokens, E], F32, isOutput=True)
    out_idx = nc.declare_dram_parameter("out_idx", [tc_tokens, TOPK], U32, isOutput=True)
    out_w = nc.declare_dram_parameter("out_w", [tc_tokens, TOPK], F32, isOutput=True)

    n_groups = tc_tokens // tgroup
    subs = tgroup // 128

    with TileContext(nc) as tc:
        with (
            tc.tile_pool(name="const", bufs=1) as cpool,
            tc.tile_pool(name="xs", bufs=2) as xpool,
            tc.tile_pool(name="ps", bufs=5, space="PSUM") as ppool,
            tc.tile_pool(name="work", bufs=3) as spool,
        ):
            # K is processed in chunks so matmuls can start as soon as the
            # first slices of the weights and x land, instead of waiting for
            # whole-tensor DMAs (~15 MB) to finish.
            # Prologue: interleave weight-chunk and group-0 x-chunk DMAs so
            # the first matmul's pair (whi_c0 + xghi_c0) lands first; bias and
            # iota (needed only by the DVE stage) go last.
            # whi and wlo are packed side by side in one tile so a single
            # N=512 matmul covers both hi passes for a given xhi stationary
            # (halves summed after accumulation; saves 1/3 of the LDWEIGHTS).
            w_ch = []
            for c in range(NCH):
                w_ch.append(cpool.tile([128, CK, 2 * E], F16, tag=f"w{c}", name=f"w_ch{c}"))

            def load_w_chunk(c):
                nc.sync.dma_start(
                    out=w_ch[c][:, :, 0:E],
                    in_=whi[ds(c * CK * 128, CK * 128), :].rearrange("(k p) e -> p k e", p=128))
                nc.sync.dma_start(
                    out=w_ch[c][:, :, E : 2 * E],
                    in_=wlo[ds(c * CK * 128, CK * 128), :].rearrange("(k p) e -> p k e", p=128))

            def load_x_chunk(g, c):
                xh = xpool.tile([128, CK, tgroup], F16, tag=f"xghi{c}")
                nc.sync.dma_start(
                    out=xh,
                    in_=xhi[ds(c * CK * 128, CK * 128), ds(g * tgroup, tgroup)]
                    .rearrange("(k p) t -> p k t", p=128))
                xl = xpool.tile([128, CK, tgroup], F16, tag=f"xglo{c}")
                nc.sync.dma_start(
                    out=xl,
                    in_=xlo[ds(c * CK * 128, CK * 128), ds(g * tgroup, tgroup)]
                    .rearrange("(k p) t -> p k t", p=128))
                return xh, xl

            # Weight chunks run 3 ahead of group-0 x chunks: the PE can
            # start after ~4 chunks (~11us) while the 3-chunk lead absorbs
            # DMA arrival jitter (starting with no lead stutters the PE and
            # re-throttles HAM; waiting for all weights idles it for 37us).
            W_LEAD = 3
            for c in range(W_LEAD):
                load_w_chunk(c)
            g0_chunks = []
            for c in range(NCH):
                g0_chunks.append(load_x_chunk(0, c))
                if c + W_LEAD < NCH:
                    load_w_chunk(c + W_LEAD)
            bias_sb = cpool.tile([128, E], F32)
            nc.sync.dma_start(out=bias_sb, in_=biasb[:, :])
            iota_sb = cpool.tile([128, E], F32)
            nc.sync.dma_start(out=iota_sb, in_=iotab[:, :])

            for g in range(n_groups):
                if g == 0:
                    xgpairs = g0_chunks
                else:
                    xgpairs = [load_x_chunk(g, c) for c in range(NCH)]
                xghi_ch = [p[0] for p in xgpairs]
                xglo_ch = [p[1] for p in xgpairs]
                for sub in range(subs):
                    t0 = g * tgroup + sub * 128
                    ts_ = ds(sub * 128, 128)
                    lg = ppool.tile([128, 2 * E], F32, tag="lg")
                    for k in range(KH):
                        c, kc = divmod(k, CK)
                        nc.tensor.matmul(lg, xghi_ch[c][:, kc, ts_], w_ch[c][:, kc, :],
                                         start=(k == 0), stop=False)
                        nc.tensor.matmul(lg[:, 0:E], xglo_ch[c][:, kc, ts_],
                                         w_ch[c][:, kc, 0:E],
                                         start=False, stop=(k == KH - 1))
                    # halves: lg[:,0:E] = xhi*whi + xlo*whi, lg[:,E:] = xhi*wlo
                    # (one PSUM operand per DVE op: stage the hi-lo half first)
                    lcor = spool.tile([128, E], F32, tag="lcor")
                    nc.scalar.copy(lcor, lg[:, E : 2 * E])
                    lsum = spool.tile([128, E], F32, tag="lsum")
                    nc.vector.tensor_add(lsum, lg[:, 0:E], lcor)
                    # descaled router logits: SBUF -> DRAM
                    lsb = spool.tile([128, E], F32, tag="lsb")
                    nc.scalar.mul(lsb, lsum, DESCALE)
                    nc.scalar.dma_start(out=out_logits[ds(t0, 128), :], in_=lsb)
                    scores = spool.tile([128, E], F32, tag="scores")
                    nc.scalar.activation(scores, lsum, mybir.ActivationFunctionType.Sigmoid,
                                         scale=DESCALE)
                    s4c = spool.tile([128, E], F32, tag="s4c")
                    nc.vector.tensor_add(s4c, scores, bias_sb)

                    gtop = spool.tile([128, NG, 8], F32, tag="gtop")
                    for j in range(NG):
                        nc.vector.max(out=gtop[:, j, :], in_=s4c[:, ds(j * GS, GS)])
                    gsum = spool.tile([128, NG], F32, tag="gsum")
                    nc.vector.tensor_add(gsum, gtop[:, :, 0], gtop[:, :, 1])
                    gs8 = spool.tile([128, 8], F32, tag="gs8")
                    nc.vector.max(out=gs8, in_=gsum)
                    gmask = spool.tile([128, NG], F32, tag="gmask")
                    nc.vector.tensor_scalar(
                        gmask, gsum, gs8[:, 3:4], None, op0=mybir.AluOpType.is_ge
                    )
                    masked = spool.tile([128, E], F32, tag="masked")
                    nc.vector.tensor_tensor(
                        out=masked.rearrange("p (g s) -> p g s", g=NG),
                        in0=s4c.rearrange("p (g s) -> p g s", g=NG),
                        in1=gmask[:, :, None].to_broadcast([128, NG, GS]),
                        op=mybir.AluOpType.mult,
                    )
                    top8v = spool.tile([128, 8], F32, tag="top8v")
                    nc.vector.max(out=top8v, in_=masked)
                    idx8 = spool.tile([128, 8], U32, tag="idx8")
                    nc.vector.max_index(idx8, top8v, masked)
                    idx8f = spool.tile([128, TOPK], F32, tag="idx8f")
                    nc.vector.tensor_copy(idx8f, idx8)
                    w8 = spool.tile([128, TOPK], F32, tag="w8")
                    eqs = spool.tile([128, E], F32, tag="eqs")
                    for k in range(TOPK):
                        nc.vector.scalar_tensor_tensor(
                            out=eqs,
                            in0=iota_sb,
                            scalar=idx8f[:, k : k + 1],
                            in1=scores,
                            op0=mybir.AluOpType.is_equal,
                            op1=mybir.AluOpType.mult,
                            accum_out=w8[:, k : k + 1],
                        )
                    denom = spool.tile([128, 1], F32, tag="denom")
                    nc.vector.reduce_sum(denom, w8, axis=mybir.AxisListType.X)
                    rden = spool.tile([128, 1], F32, tag="rden")
                    nc.vector.reciprocal(rden, denom)
                    wout = spool.tile([128, TOPK], F32, tag="wout")
                    nc.vector.tensor_scalar(
                        wout, w8, rden, SCALE,
                        op0=mybir.AluOpType.mult, op1=mybir.AluOpType.mult,
                    )
                    nc.scalar.dma_start(out=out_idx[ds(t0, 128), :], in_=idx8)
                    nc.scalar.dma_start(out=out_w[ds(t0, 128), :], in_=wout)
    nc.finalize()
    return nc


def make_in_maps_f16(hidden_states, weight, e_score_correction_bias):
    from concurrent.futures import ThreadPoolExecutor

    x = np.ascontiguousarray(np.asarray(hidden_states, dtype=np.float32)).reshape(T, H)
    w = np.asarray(weight, dtype=np.float32)
    b = np.asarray(e_score_correction_bias, dtype=np.float32)
    ws = np.ascontiguousarray(w.T) * np.float32(WSCALE)
    whi = ws.astype(np.float16)
    wlo = (ws - whi.astype(np.float32)).astype(np.float16)
    biasb = np.ascontiguousarray(np.broadcast_to(b[None, :], (128, E)))
    iotab = np.ascontiguousarray(
        np.broadcast_to(np.arange(E, dtype=np.float32)[None, :], (128, E)))
    xt_full = x.T  # view

    def prep(c):
        xs = np.ascontiguousarray(xt_full[:, c * TC : (c + 1) * TC]) * np.float32(XSCALE)
        xhi = xs.astype(np.float16)
        xlo = (xs - xhi.astype(np.float32)).astype(np.float16)
        return {"xhi": xhi, "xlo": xlo, "whi": whi, "wlo": wlo,
                "biasb": biasb, "iotab": iotab}

    with ThreadPoolExecutor(N_CORES) as ex:
        return list(ex.map(prep, range(N_CORES)))


_NC = None


def _get_nc():
    global _NC
    if _NC is None:
        _NC = build_nc_f16()
    return _NC


def make_in_maps(hidden_states, weight, e_score_correction_bias):
    x = np.ascontiguousarray(np.asarray(hidden_states, dtype=np.float32)).reshape(T, H)
    w = np.asarray(weight, dtype=np.float32)
    b = np.asarray(e_score_correction_bias, dtype=np.float32)
    wt = np.ascontiguousarray(w.T)
    biasb = np.ascontiguousarray(np.broadcast_to(b[None, :], (128, E)))
    iotab = np.ascontiguousarray(np.broadcast_to(np.arange(E, dtype=np.float32)[None, :], (128, E)))
    xt_full = x.T  # view
    in_maps = []
    for c in range(N_CORES):
        xt_c = np.ascontiguousarray(xt_full[:, c * TC : (c + 1) * TC])
        in_maps.append({"xt": xt_c, "wt": wt, "biasb": biasb, "iotab": iotab})
    return in_maps


def assemble(results):
    logits = np.concatenate([results[c]["out_logits"] for c in range(N_CORES)], axis=0)
    idx = np.concatenate([results[c]["out_idx"] for c in range(N_CORES)], axis=0).astype(np.int32)
    wts = np.concatenate([results[c]["out_w"] for c in range(N_CORES)], axis=0)
    return idx, wts, logits


make_in_maps_active = None  # set below


def kernel(hidden_states, weight, e_score_correction_bias):
    nc = _get_nc()
    in_maps = make_in_maps_active(hidden_states, weight, e_score_correction_bias)
    res = run_bass_kernel_spmd(nc, in_maps, list(range(N_CORES)))
    return assemble(res.results)


make_in_maps_active = make_in_maps_f16
